# revision 28
# baseline (speedup 1.0000x reference)
"""Tensor-parallel GQA multi-head-attention kernel for 8 trn2 NeuronCores.

Problem: B=2, T=2048, D=2048, H=16 q-heads, KV=4 kv-heads, HD=128,
causal attention with interleaved RoPE, y = attn_out @ Wo.

Sharding (tensor-parallel over heads, per the hint):
  core c = b*4 + g   (b = batch index, g = kv-head / q-head-group index)
  Each core computes q-heads 4g..4g+3 and kv-head g for batch b, plus the
  partial output  y_partial = attn_heads @ Wo[rows of those heads]  (row-
  parallel Wo).  The host sums the 4 partials per batch (the unshard of the
  row-parallel all-reduce) and stacks the 2 batches.

On-chip design (per core, everything bf16 except PSUM/softmax math):
  - host pre-arranges every input partition-major ([128, k, ...]) so each
    tensor loads with a handful of large contiguous DMAs, issued in exact
    consumption order on two HW DGE queues (sync + scalar).
  - projections k-outer: per contraction chunk k, 4 q-head matmuls (psum
    banks qp0..qp3) stream xT columns; then k-proj, then v-proj (natural
    layout, 4 T-tiles packed in one psum bank).
  - RoPE: rot = q*cos_dup + swap(q)*[-sin|sin]; psum->bf16 copy on ACT,
    half-swap via SBUF->SBUF DMA (gpsimd queue), muls on DVE in bf16.
  - attention per (head, 512-wide q chunk): for each 128-row k tile
    S^T = k^T_tile.T @ q^T chunk -> PSUM [128,512]; diagonal blocks get the
    -30000 mask PRE-loaded into psum (DVE copy + start=False matmul) so the
    S->exp->PV chain has no DVE hop; ACT computes P = exp(scale*S^T) -> bf16;
    PV accumulates out^T[HD,512]; an all-ones [128,128] lhsT matmul
    accumulates softmax denominators; normalization = reciprocal + DVE mul.
    Fully-masked (future) blocks are skipped.  S matmuls are emitted two
    tiles ahead of PV so the PE never waits on the ACT exp.
  - Wo: y tile [128,512] = sum_h attnT_h chunk.T @ Wo_h chunk, psum->bf16
    copy alternating ACT/DVE, bf16 DMA to DRAM on the scalar queue.
    Host sums the 4 bf16 partials per batch in f32.
"""

import math
import sys

import numpy as np

for _p in ("/opt/trn_rl_repo", "/root/.axon_site",
           "/root/.axon_site/_ro/trn_rl_repo",
           "/root/.axon_site/_ro/pypackages"):
    if _p not in sys.path:
        sys.path.append(_p)

B, T, D = 2, 2048, 2048
H, KV, HD = 16, 4, 128
ROPE_BASE = 10000.0
N_CORES = 8
HEADS_PER_CORE = 4
DQ = HEADS_PER_CORE * HD  # 512 q-dims per core
SCALE = 1.0 / math.sqrt(HD)
MASK_VAL = -30000.0

_CACHE = {}


def _build_nc(t_len=T):
    """Build the single-core SPMD Bass/Tile program (cached)."""
    import concourse.bass as bass
    import concourse.mybir as mybir
    import concourse.tile as tile
    from concourse import bacc

    f32 = mybir.dt.float32
    bf16 = mybir.dt.bfloat16
    ts = bass.ts

    NT = t_len // 128        # number of 128-row T tiles
    NK = D // 128            # contraction chunks for projections
    NCQ = t_len // 512       # number of 512-wide q chunks

    nc = bacc.Bacc("TRN2", target_bir_lowering=False, debug=False,
                   num_devices=N_CORES)

    # All inputs host-pre-arranged partition-major.
    xT_d = nc.dram_tensor("xT", [128, NK, t_len], bf16, kind="ExternalInput").ap()
    wq_d = nc.dram_tensor("wq", [128, NK, DQ], bf16, kind="ExternalInput").ap()
    wk_d = nc.dram_tensor("wk", [128, NK, HD], bf16, kind="ExternalInput").ap()
    wv_d = nc.dram_tensor("wv", [128, NK, HD], bf16, kind="ExternalInput").ap()
    wo_d = nc.dram_tensor("wo", [128, HEADS_PER_CORE, D], bf16,
                          kind="ExternalInput").ap()
    cos_d = nc.dram_tensor("cosd", [128, t_len], bf16, kind="ExternalInput").ap()
    ssig_d = nc.dram_tensor("ssig", [128, t_len], bf16, kind="ExternalInput").ap()
    mask_d = nc.dram_tensor("mask", [128, 128], bf16, kind="ExternalInput").ap()
    ident_d = nc.dram_tensor("ident", [128, 128], bf16, kind="ExternalInput").ap()
    y_d = nc.dram_tensor("y", [t_len, D], bf16, kind="ExternalOutput").ap()

    Exp = mybir.ActivationFunctionType.Exp

    with tile.TileContext(nc) as tc:
        with (
            tc.tile_pool(name="const", bufs=1) as const,
            tc.tile_pool(name="qkv", bufs=1) as qkv,
            tc.tile_pool(name="attn", bufs=2) as attn_pool,
            tc.tile_pool(name="p", bufs=6) as p_pool,
            tc.tile_pool(name="rope", bufs=3) as rope_pool,
            tc.tile_pool(name="recip", bufs=2) as recip_pool,
            tc.tile_pool(name="y", bufs=4) as y_pool,
            tc.tile_pool(name="psum", bufs=1, space="PSUM") as psum,
        ):
            # ---- persistent input tiles ----
            xT = const.tile([128, NK, t_len], bf16, tag="xT")
            wq = const.tile([128, NK, DQ], bf16, tag="wq")
            wk = const.tile([128, NK, HD], bf16, tag="wk")
            wv = const.tile([128, NK, HD], bf16, tag="wv")
            wo = const.tile([128, HEADS_PER_CORE, D], bf16, tag="wo")
            cos_sb = const.tile([128, t_len], bf16, tag="cos")
            ssig_sb = const.tile([128, t_len], bf16, tag="ssig")
            mask_sb = const.tile([128, 128], bf16, tag="mask")
            ident_sb = const.tile([128, 128], bf16, tag="ident")
            ones_sb = const.tile([128, 128], bf16, tag="ones")

            # ---- input DMA streams in exact consumption order ----
            # sync queue: q-proj consumes (wq, xT chunk-0 cols) quad pairs.
            # DMA-issue instructions cost ~1us each, so quad granularity
            # (512KB) keeps issue rate above the HBM transfer rate.
            for ks in (slice(0, 2), slice(2, 4), slice(4, 8),
                       slice(8, 12), slice(12, 16)):
                nc.sync.dma_start(wq[:, ks, :], wq_d[:, ks, :])
                nc.sync.dma_start(xT[:, ks, 0:512], xT_d[:, ks, 0:512])
            # scalar queue: only wk is needed early (k-proj is interleaved
            # into the chunk-0 quad loop); the other small inputs are issued
            # after the quad stream so they don't compete for HBM bandwidth
            # during the startup-critical window.
            nc.scalar.dma_start(wk[:], wk_d[:])
            nc.scalar.dma_start(cos_sb[:, 0:512], cos_d[:, 0:512])
            nc.scalar.dma_start(ssig_sb[:, 0:512], ssig_d[:, 0:512])
            nc.scalar.dma_start(wv[:], wv_d[:])
            nc.scalar.dma_start(mask_sb[:], mask_d[:])
            nc.scalar.dma_start(ident_sb[:], ident_d[:])
            nc.vector.memset(ones_sb[:], 1.0)

            # persistent activations
            qT = qkv.tile([128, HEADS_PER_CORE, t_len], bf16, tag="qT")
            kT = qkv.tile([128, t_len], bf16, tag="kT")
            v_sb = qkv.tile([128, NT, HD], bf16, tag="v")

            swap_ctr = [0]

            def rope_pre(psum_tile):
                """psum -> bf16 copy (ACT) + half-swap DMAs; returns the
                (qf, qs) tiles for a deferred rope_muls call.  Swap DMAs
                alternate between the gpsimd and sync queues so a chunk's
                rope chain doesn't serialize on one DMA-issue engine."""
                qf = rope_pool.tile([128, 512], bf16, tag="qf", bufs=6)
                nc.scalar.copy(qf[:], psum_tile[:])
                qs = rope_pool.tile([128, 512], bf16, tag="qs", bufs=6)
                eng = nc.gpsimd if swap_ctr[0] % 2 == 0 else nc.sync
                swap_ctr[0] += 1
                eng.dma_start(qs[0:64, :], qf[64:128, :])
                eng.dma_start(qs[64:128, :], qf[0:64, :])
                return qf, qs

            def rope_muls(dst_ap, qf, qs, c):
                cs = slice(c * 512, (c + 1) * 512)
                nc.vector.tensor_mul(qf[:], qf[:], cos_sb[:, cs])
                nc.vector.tensor_mul(qs[:], qs[:], ssig_sb[:, cs])
                nc.vector.tensor_add(dst_ap, qf[:], qs[:])

            def rope_to(dst_ap, psum_tile, c):
                qf, qs = rope_pre(psum_tile)
                rope_muls(dst_ap, qf, qs, c)

            def q_head(c, h, qp_h, k_range):
                cs = slice(c * 512, (c + 1) * 512)
                for k in k_range:
                    nc.tensor.matmul(qp_h[:], wq[:, k, ts(h, 128)],
                                     xT[:, k, cs],
                                     start=(k == 0), stop=(k == NK - 1))

            def k_proj(c):
                cs = slice(c * 512, (c + 1) * 512)
                kp = psum.tile([128, 512], f32, tag="B", bufs=4)
                for k in range(NK):
                    nc.tensor.matmul(kp[:], wk[:, k, :], xT[:, k, cs],
                                     start=(k == 0), stop=(k == NK - 1))
                rope_to(kT[:, cs], kp, c)

            def v_proj(c):
                vp = psum.tile([128, 4, HD], f32, tag="B", bufs=4)
                for tt in range(4):
                    tcols = ts(4 * c + tt, 128)
                    for k in range(NK):
                        nc.tensor.matmul(vp[:, tt, :], xT[:, k, tcols],
                                         wv[:, k, :],
                                         start=(k == 0), stop=(k == NK - 1))
                return vp

            def proj_chunk(c):
                """Projections for T positions [c*512, (c+1)*512).

                Chunk 0 runs q-proj k-outer so PE consumption stays below
                DMA delivery; later chunks run k-proj first and q-proj
                head-by-head so each head's rope chain overlaps the
                remaining projection matmuls instead of serializing at the
                chunk boundary."""
                cs = slice(c * 512, (c + 1) * 512)
                if c == 0:
                    # k-outer with k-proj interleaved: PE consumes the DMA
                    # stream at ~235GB/s < delivery rate, so no starvation.
                    # Last quad: k-proj finishes first (rope_k emitted), then
                    # q heads finish one-by-one so their ropes pipeline.
                    qp = [psum.tile([128, 512], f32, tag="A", bufs=4,
                                    name=f"qp{h}")
                          for h in range(HEADS_PER_CORE)]
                    kp = psum.tile([128, 512], f32, tag="B", bufs=4)
                    for k in range(12):
                        for h in range(HEADS_PER_CORE):
                            nc.tensor.matmul(qp[h][:], wq[:, k, ts(h, 128)],
                                             xT[:, k, cs],
                                             start=(k == 0), stop=False)
                        nc.tensor.matmul(kp[:], wk[:, k, :], xT[:, k, cs],
                                         start=(k == 0), stop=False)
                    for k in range(12, NK):
                        nc.tensor.matmul(kp[:], wk[:, k, :], xT[:, k, cs],
                                         start=False, stop=(k == NK - 1))
                    rope_to(kT[:, cs], kp, c)
                    # heads 1-3: defer the DVE muls until after the v copy
                    # so the attention PV matmuls aren't blocked behind 15
                    # serial rope muls in the DVE queue.
                    deferred = []
                    for h in range(HEADS_PER_CORE):
                        for k in range(12, NK):
                            nc.tensor.matmul(qp[h][:], wq[:, k, ts(h, 128)],
                                             xT[:, k, cs],
                                             start=False, stop=(k == NK - 1))
                        if h == 0:
                            rope_to(qT[:, h, cs], qp[h], c)
                        else:
                            deferred.append((h, rope_pre(qp[h])))
                    vp = v_proj(c)
                    nc.vector.tensor_copy(v_sb[:, 0:4, :], vp[:])
                    for h, (qf, qs) in deferred:
                        rope_muls(qT[:, h, cs], qf, qs, c)
                    return
                else:
                    k_proj(c)
                    for h in range(HEADS_PER_CORE):
                        qp_h = psum.tile([128, 512], f32, tag="A", bufs=4,
                                         name=f"qp{h}")
                        q_head(c, h, qp_h, range(NK))
                        rope_to(qT[:, h, cs], qp_h, c)
                    vp = v_proj(c)
                nc.vector.tensor_copy(v_sb[:, 4 * c:4 * c + 4, :], vp[:])

            def attn_chunk(c):
                """Attention for q chunk c (all 4 heads) -> attnT tile."""
                attn_t = attn_pool.tile([128, HEADS_PER_CORE, 512], bf16,
                                        tag="attnT")
                nj = 4 * c + 4

                def emit_s(h, j):
                    """S^T matmul + exp for (head h, key tile j); diagonal
                    blocks get the mask added by a tiny extra matmul in the
                    same accumulation group (identity.T @ mask = mask), so
                    the S->exp chain never leaves the PE/ACT engines."""
                    o = j - 4 * c
                    lo = max(o, 0) * 128
                    q0 = c * 512
                    s_ps = psum.tile([128, 512], f32, tag="A", bufs=4,
                                     name="s_ps")
                    if o >= 0:
                        nc.tensor.matmul(s_ps[:, lo:lo + 128],
                                         kT[:, ts(j, 128)],
                                         qT[:, h, q0 + lo:q0 + lo + 128],
                                         start=True, stop=False)
                        nc.tensor.matmul(s_ps[:, lo:lo + 128], ident_sb[:],
                                         mask_sb[:], start=False, stop=True)
                        if lo + 128 < 512:
                            nc.tensor.matmul(s_ps[:, lo + 128:],
                                             kT[:, ts(j, 128)],
                                             qT[:, h, q0 + lo + 128:q0 + 512],
                                             start=True, stop=True)
                    else:
                        nc.tensor.matmul(s_ps[:], kT[:, ts(j, 128)],
                                         qT[:, h, q0:q0 + 512],
                                         start=True, stop=True)
                    p = p_pool.tile([128, 512], bf16, tag="p")
                    nc.scalar.activation(p[:, lo:], s_ps[:, lo:], Exp,
                                         bias=0.0, scale=SCALE)
                    return p, lo

                # late input loads on the scalar queue, issued while this
                # chunk's attention runs: wo (needed from wo_chunk(0)) and
                # the xT columns / rope tables of chunk c+1.
                if c == 0:
                    for h in range(HEADS_PER_CORE):
                        nc.scalar.dma_start(wo[:, h, :], wo_d[:, h, :])
                if c + 1 < NCQ:
                    ns = slice((c + 1) * 512, (c + 2) * 512)
                    nc.scalar.dma_start(xT[:, :, ns], xT_d[:, :, ns])
                    nc.scalar.dma_start(cos_sb[:, ns], cos_d[:, ns])
                    nc.scalar.dma_start(ssig_sb[:, ns], ssig_d[:, ns])
                for h in range(HEADS_PER_CORE):
                    out_ps = psum.tile([128, 512], f32, tag="B", bufs=4)
                    sums_ps = psum.tile([128, 512], f32, tag="B", bufs=4)
                    pend = [emit_s(h, j) for j in range(min(2, nj))]
                    for j in range(nj):
                        if j + 2 < nj:
                            pend.append(emit_s(h, j + 2))
                        p, lo = pend.pop(0)
                        nc.tensor.matmul(out_ps[:, lo:], v_sb[:, j, :],
                                         p[:, lo:],
                                         start=(j == 0), stop=(j == nj - 1))
                        nc.tensor.matmul(sums_ps[:, lo:], ones_sb[:],
                                         p[:, lo:],
                                         start=(j == 0), stop=(j == nj - 1))
                    rc = recip_pool.tile([128, 512], f32, tag="rc")
                    nc.vector.reciprocal_approx_fast(out=rc[:], in_=sums_ps[:])
                    nc.vector.tensor_mul(attn_t[:, h, :], out_ps[:], rc[:])
                return attn_t

            def wo_chunk(c, attn_t):
                """Output projection for q chunk c; bf16 partials to DRAM."""
                for tq in range(4):
                    row0 = (4 * c + tq) * 128
                    for nn in range(4):
                        yp = psum.tile([128, 512], f32, tag="A", bufs=4)
                        for h in range(HEADS_PER_CORE):
                            nc.tensor.matmul(yp[:],
                                             attn_t[:, h, ts(tq, 128)],
                                             wo[:, h, ts(nn, 512)],
                                             start=(h == 0), stop=(h == 3))
                        ysb = y_pool.tile([128, 512], bf16, tag="y")
                        if nn % 2 == 0:
                            nc.scalar.copy(ysb[:], yp[:])
                        else:
                            nc.vector.tensor_copy(ysb[:], yp[:])
                        nc.sync.dma_start(
                            y_d[row0:row0 + 128, ts(nn, 512)], ysb[:])

            # Phase order p0 a0 [p1] w0 a1 [p2] w1 a2 [p3] w2 a3 w3:
            # proj(c+1) has no dependency on attn(c), so emitting it before
            # wo(c) gives the PE ~20us of independent work while the last
            # head's softmax normalization (ACT exp tail + DVE recip/mul)
            # completes -- wo(c) then starts with attn_t fully ready.
            proj_chunk(0)
            for c in range(NCQ):
                at = attn_chunk(c)
                if c + 1 < NCQ:
                    proj_chunk(c + 1)
                wo_chunk(c, at)

    nc.finalize()
    return nc


def _to_pmajor(a, nk):
    """[nk*128, F] -> [128, nk, F] partition-major layout."""
    f = a.shape[1]
    return np.ascontiguousarray(a.reshape(nk, 128, f).transpose(1, 0, 2))


def _prep_inputs(x, Wq, Wk, Wv, Wo, t_len=T):
    """Host-side shard + layout prep -> per-core input maps."""
    import ml_dtypes
    bf16 = ml_dtypes.bfloat16

    x = np.asarray(x, np.float32)
    Wq = np.asarray(Wq, np.float32)
    Wk = np.asarray(Wk, np.float32)
    Wv = np.asarray(Wv, np.float32)
    Wo = np.asarray(Wo, np.float32)

    NK = D // 128

    # RoPE de-interleave permutation within one head: [evens | odds]
    perm = np.concatenate([np.arange(0, HD, 2), np.arange(1, HD, 2)])

    # rope tables (match reference: freqs = t * base**(-2j/HD))
    inv = 1.0 / (ROPE_BASE ** (np.arange(0, HD, 2, dtype=np.float32) / HD))
    tpos = np.arange(t_len, dtype=np.float32)
    f = inv[:, None] * tpos[None, :]                       # [64, T]
    cos_dup = np.concatenate([np.cos(f), np.cos(f)], 0)    # [128, T]
    ssig = np.concatenate([-np.sin(f), np.sin(f)], 0)      # [128, T]
    cos_dup = cos_dup.astype(bf16)
    ssig = ssig.astype(bf16)

    # strict-lower-triangular causal mask template for the diagonal
    # [tk-tile, tq-tile] block (tk > tq within the 128x128 block)
    r = np.arange(128)[:, None]
    col = np.arange(128)[None, :]
    mask_t = np.where(r > col, MASK_VAL, 0.0).astype(bf16)
    ident = np.eye(128, dtype=np.float32).astype(bf16)

    in_maps = []
    for b in range(B):
        xT_b = _to_pmajor(np.ascontiguousarray(x[b, :t_len].T),
                          NK).astype(bf16)               # [128, NK, T]
        for g in range(KV):
            wq_g = Wq[:, g * DQ:(g + 1) * DQ].reshape(D, HEADS_PER_CORE, HD)
            wq_g = _to_pmajor(np.ascontiguousarray(
                wq_g[:, :, perm].reshape(D, DQ)), NK).astype(bf16)
            wk_g = _to_pmajor(np.ascontiguousarray(
                Wk[:, g * HD:(g + 1) * HD][:, perm]), NK).astype(bf16)
            wv_g = _to_pmajor(np.ascontiguousarray(
                Wv[:, g * HD:(g + 1) * HD]), NK).astype(bf16)
            wo_g = _to_pmajor(np.ascontiguousarray(
                Wo[g * DQ:(g + 1) * DQ, :]), HEADS_PER_CORE).astype(bf16)
            in_maps.append({
                "xT": xT_b, "wq": wq_g, "wk": wk_g, "wv": wv_g,
                "wo": wo_g, "cosd": cos_dup, "ssig": ssig, "mask": mask_t,
                "ident": ident,
            })
    return in_maps


def run(inputs, trace=False, t_len=T):
    """Run the sharded kernel; returns (y_full, BassKernelResults)."""
    from concourse.bass_utils import run_bass_kernel_spmd

    key = ("nc", t_len)
    if key not in _CACHE:
        _CACHE[key] = _build_nc(t_len)
    nc = _CACHE[key]

    in_maps = _prep_inputs(inputs["x"], inputs["Wq"], inputs["Wk"],
                           inputs["Wv"], inputs["Wo"], t_len)
    res = run_bass_kernel_spmd(nc, in_maps, list(range(N_CORES)), trace=trace)

    y = np.empty((B, t_len, D), np.float32)
    for b in range(B):
        acc = np.zeros((t_len, D), np.float32)
        for g in range(KV):
            acc += np.asarray(res.results[b * KV + g]["y"], np.float32)
        y[b] = acc
    return y, res


def kernel(**inputs) -> np.ndarray:
    y, _ = run(inputs, trace=False)
    return y


# revision 30
# speedup vs baseline: 1.0047x; 1.0047x over previous
"""Tensor-parallel GQA multi-head-attention kernel for 8 trn2 NeuronCores.

Problem: B=2, T=2048, D=2048, H=16 q-heads, KV=4 kv-heads, HD=128,
causal attention with interleaved RoPE, y = attn_out @ Wo.

Sharding (tensor-parallel over heads, per the hint):
  core c = b*4 + g   (b = batch index, g = kv-head / q-head-group index)
  Each core computes q-heads 4g..4g+3 and kv-head g for batch b, plus the
  partial output  y_partial = attn_heads @ Wo[rows of those heads]  (row-
  parallel Wo).  The host sums the 4 partials per batch (the unshard of the
  row-parallel all-reduce) and stacks the 2 batches.

On-chip design (per core, everything bf16 except PSUM/softmax math):
  - host pre-arranges every input partition-major ([128, k, ...]) so each
    tensor loads with a handful of large contiguous DMAs, issued in exact
    consumption order on two HW DGE queues (sync + scalar).
  - projections k-outer: per contraction chunk k, 4 q-head matmuls (psum
    banks qp0..qp3) stream xT columns; then k-proj, then v-proj (natural
    layout, 4 T-tiles packed in one psum bank).
  - RoPE: rot = q*cos_dup + swap(q)*[-sin|sin]; psum->bf16 copy on ACT,
    half-swap via SBUF->SBUF DMA (gpsimd queue), muls on DVE in bf16.
  - attention per (head, 512-wide q chunk): for each 128-row k tile
    S^T = k^T_tile.T @ q^T chunk -> PSUM [128,512]; diagonal blocks get the
    -30000 mask PRE-loaded into psum (DVE copy + start=False matmul) so the
    S->exp->PV chain has no DVE hop; ACT computes P = exp(scale*S^T) -> bf16;
    PV accumulates out^T[HD,512]; an all-ones [128,128] lhsT matmul
    accumulates softmax denominators; normalization = reciprocal + DVE mul.
    Fully-masked (future) blocks are skipped.  S matmuls are emitted two
    tiles ahead of PV so the PE never waits on the ACT exp.
  - Wo: y tile [128,512] = sum_h attnT_h chunk.T @ Wo_h chunk, psum->bf16
    copy alternating ACT/DVE, bf16 DMA to DRAM on the scalar queue.
    Host sums the 4 bf16 partials per batch in f32.
"""

import math
import sys

import numpy as np

for _p in ("/opt/trn_rl_repo", "/root/.axon_site",
           "/root/.axon_site/_ro/trn_rl_repo",
           "/root/.axon_site/_ro/pypackages"):
    if _p not in sys.path:
        sys.path.append(_p)

B, T, D = 2, 2048, 2048
H, KV, HD = 16, 4, 128
ROPE_BASE = 10000.0
N_CORES = 8
HEADS_PER_CORE = 4
DQ = HEADS_PER_CORE * HD  # 512 q-dims per core
SCALE = 1.0 / math.sqrt(HD)
MASK_VAL = -30000.0

_CACHE = {}


def _build_nc(t_len=T):
    """Build the single-core SPMD Bass/Tile program (cached)."""
    import concourse.bass as bass
    import concourse.mybir as mybir
    import concourse.tile as tile
    from concourse import bacc

    f32 = mybir.dt.float32
    bf16 = mybir.dt.bfloat16
    ts = bass.ts

    NT = t_len // 128        # number of 128-row T tiles
    NK = D // 128            # contraction chunks for projections
    NCQ = t_len // 512       # number of 512-wide q chunks

    nc = bacc.Bacc("TRN2", target_bir_lowering=False, debug=False,
                   num_devices=N_CORES)

    # All inputs host-pre-arranged partition-major.
    xT_d = nc.dram_tensor("xT", [128, NK, t_len], bf16, kind="ExternalInput").ap()
    wq_d = nc.dram_tensor("wq", [128, NK, DQ], bf16, kind="ExternalInput").ap()
    wk_d = nc.dram_tensor("wk", [128, NK, HD], bf16, kind="ExternalInput").ap()
    wv_d = nc.dram_tensor("wv", [128, NK, HD], bf16, kind="ExternalInput").ap()
    wo_d = nc.dram_tensor("wo", [128, HEADS_PER_CORE, D], bf16,
                          kind="ExternalInput").ap()
    cos_d = nc.dram_tensor("cosd", [128, t_len], bf16, kind="ExternalInput").ap()
    ssig_d = nc.dram_tensor("ssig", [128, t_len], bf16, kind="ExternalInput").ap()
    mask_d = nc.dram_tensor("mask", [128, 128], bf16, kind="ExternalInput").ap()
    ident_d = nc.dram_tensor("ident", [128, 128], bf16, kind="ExternalInput").ap()
    y_d = nc.dram_tensor("y", [t_len, D], bf16, kind="ExternalOutput").ap()

    Exp = mybir.ActivationFunctionType.Exp

    with tile.TileContext(nc) as tc:
        with (
            tc.tile_pool(name="const", bufs=1) as const,
            tc.tile_pool(name="qkv", bufs=1) as qkv,
            tc.tile_pool(name="attn", bufs=2) as attn_pool,
            tc.tile_pool(name="p", bufs=6) as p_pool,
            tc.tile_pool(name="rope", bufs=3) as rope_pool,
            tc.tile_pool(name="recip", bufs=2) as recip_pool,
            tc.tile_pool(name="y", bufs=4) as y_pool,
            tc.tile_pool(name="psum", bufs=1, space="PSUM") as psum,
        ):
            # ---- persistent input tiles ----
            xT = const.tile([128, NK, t_len], bf16, tag="xT")
            wq = const.tile([128, NK, DQ], bf16, tag="wq")
            wk = const.tile([128, NK, HD], bf16, tag="wk")
            wv = const.tile([128, NK, HD], bf16, tag="wv")
            wo = const.tile([128, HEADS_PER_CORE, D], bf16, tag="wo")
            cos_sb = const.tile([128, t_len], bf16, tag="cos")
            ssig_sb = const.tile([128, t_len], bf16, tag="ssig")
            mask_sb = const.tile([128, 128], bf16, tag="mask")
            ident_sb = const.tile([128, 128], bf16, tag="ident")
            ones_sb = const.tile([128, 128], bf16, tag="ones")

            # ---- input DMA streams in exact consumption order ----
            # sync queue: q-proj consumes (wq, xT chunk-0 cols) quad pairs.
            # DMA-issue instructions cost ~1us each, so quad granularity
            # (512KB) keeps issue rate above the HBM transfer rate.
            for ks in (slice(0, 2), slice(2, 4), slice(4, 8),
                       slice(8, 12), slice(12, 16)):
                nc.sync.dma_start(wq[:, ks, :], wq_d[:, ks, :])
                nc.sync.dma_start(xT[:, ks, 0:512], xT_d[:, ks, 0:512])
            # scalar queue: only wk is needed early (k-proj is interleaved
            # into the chunk-0 quad loop); the other small inputs are issued
            # after the quad stream so they don't compete for HBM bandwidth
            # during the startup-critical window.
            nc.scalar.dma_start(wk[:], wk_d[:])
            nc.scalar.dma_start(cos_sb[:, 0:512], cos_d[:, 0:512])
            nc.scalar.dma_start(ssig_sb[:, 0:512], ssig_d[:, 0:512])
            nc.scalar.dma_start(wv[:], wv_d[:])
            nc.scalar.dma_start(mask_sb[:], mask_d[:])
            nc.scalar.dma_start(ident_sb[:], ident_d[:])
            nc.vector.memset(ones_sb[:], 1.0)

            # persistent activations
            qT = qkv.tile([128, HEADS_PER_CORE, t_len], bf16, tag="qT")
            kT = qkv.tile([128, t_len], bf16, tag="kT")
            v_sb = qkv.tile([128, NT, HD], bf16, tag="v")

            swap_ctr = [0]

            def rope_pre(psum_tile, copy_eng="scalar"):
                """psum -> bf16 copy + half-swap DMAs; returns the
                (qf, qs) tiles for a deferred rope_muls call.  Swap DMAs
                alternate between the gpsimd and sync queues so a chunk's
                rope chain doesn't serialize on one DMA-issue engine."""
                qf = rope_pool.tile([128, 512], bf16, tag="qf", bufs=6)
                if copy_eng == "scalar":
                    nc.scalar.copy(qf[:], psum_tile[:])
                else:
                    nc.vector.tensor_copy(qf[:], psum_tile[:])
                qs = rope_pool.tile([128, 512], bf16, tag="qs", bufs=6)
                eng = nc.gpsimd if swap_ctr[0] % 2 == 0 else nc.sync
                swap_ctr[0] += 1
                eng.dma_start(qs[0:64, :], qf[64:128, :])
                eng.dma_start(qs[64:128, :], qf[0:64, :])
                return qf, qs

            def rope_muls(dst_ap, qf, qs, c):
                cs = slice(c * 512, (c + 1) * 512)
                nc.vector.tensor_mul(qf[:], qf[:], cos_sb[:, cs])
                nc.vector.tensor_mul(qs[:], qs[:], ssig_sb[:, cs])
                nc.vector.tensor_add(dst_ap, qf[:], qs[:])

            def rope_to(dst_ap, psum_tile, c):
                qf, qs = rope_pre(psum_tile)
                rope_muls(dst_ap, qf, qs, c)

            def q_head(c, h, qp_h, k_range):
                cs = slice(c * 512, (c + 1) * 512)
                for k in k_range:
                    nc.tensor.matmul(qp_h[:], wq[:, k, ts(h, 128)],
                                     xT[:, k, cs],
                                     start=(k == 0), stop=(k == NK - 1))

            def k_proj(c):
                cs = slice(c * 512, (c + 1) * 512)
                kp = psum.tile([128, 512], f32, tag="B", bufs=4)
                for k in range(NK):
                    nc.tensor.matmul(kp[:], wk[:, k, :], xT[:, k, cs],
                                     start=(k == 0), stop=(k == NK - 1))
                rope_to(kT[:, cs], kp, c)

            def v_proj(c):
                vp = psum.tile([128, 4, HD], f32, tag="B", bufs=4)
                for tt in range(4):
                    tcols = ts(4 * c + tt, 128)
                    for k in range(NK):
                        nc.tensor.matmul(vp[:, tt, :], xT[:, k, tcols],
                                         wv[:, k, :],
                                         start=(k == 0), stop=(k == NK - 1))
                return vp

            def proj_chunk(c):
                """Projections for T positions [c*512, (c+1)*512).

                Chunk 0 runs q-proj k-outer so PE consumption stays below
                DMA delivery; later chunks run k-proj first and q-proj
                head-by-head so each head's rope chain overlaps the
                remaining projection matmuls instead of serializing at the
                chunk boundary."""
                cs = slice(c * 512, (c + 1) * 512)
                if c == 0:
                    # k-outer with k-proj interleaved: PE consumes the DMA
                    # stream at ~235GB/s < delivery rate, so no starvation.
                    # Last quad: k-proj finishes first (rope_k emitted), then
                    # q heads finish one-by-one so their ropes pipeline.
                    qp = [psum.tile([128, 512], f32, tag="A", bufs=4,
                                    name=f"qp{h}")
                          for h in range(HEADS_PER_CORE)]
                    kp = psum.tile([128, 512], f32, tag="B", bufs=4)
                    for k in range(12):
                        for h in range(HEADS_PER_CORE):
                            nc.tensor.matmul(qp[h][:], wq[:, k, ts(h, 128)],
                                             xT[:, k, cs],
                                             start=(k == 0), stop=False)
                        nc.tensor.matmul(kp[:], wk[:, k, :], xT[:, k, cs],
                                         start=(k == 0), stop=False)
                    for k in range(12, NK):
                        nc.tensor.matmul(kp[:], wk[:, k, :], xT[:, k, cs],
                                         start=False, stop=(k == NK - 1))
                    rope_to(kT[:, cs], kp, c)
                    # heads 1-3: defer the DVE muls until after the v copy
                    # so the attention PV matmuls aren't blocked behind 15
                    # serial rope muls in the DVE queue.
                    deferred = []
                    for h in range(HEADS_PER_CORE):
                        for k in range(12, NK):
                            nc.tensor.matmul(qp[h][:], wq[:, k, ts(h, 128)],
                                             xT[:, k, cs],
                                             start=False, stop=(k == NK - 1))
                        if h == 0:
                            rope_to(qT[:, h, cs], qp[h], c)
                        else:
                            # DVE copies: chunk-0 attention is ACT-bound
                            # (exp chain), so keep the ACT queue clear
                            deferred.append((h, rope_pre(qp[h], "vector")))
                    vp = v_proj(c)
                    nc.vector.tensor_copy(v_sb[:, 0:4, :], vp[:])
                    for h, (qf, qs) in deferred:
                        rope_muls(qT[:, h, cs], qf, qs, c)
                    return
                else:
                    k_proj(c)
                    for h in range(HEADS_PER_CORE):
                        qp_h = psum.tile([128, 512], f32, tag="A", bufs=4,
                                         name=f"qp{h}")
                        q_head(c, h, qp_h, range(NK))
                        rope_to(qT[:, h, cs], qp_h, c)
                    vp = v_proj(c)
                nc.vector.tensor_copy(v_sb[:, 4 * c:4 * c + 4, :], vp[:])

            def attn_chunk(c):
                """Attention for q chunk c (all 4 heads) -> attnT tile."""
                attn_t = attn_pool.tile([128, HEADS_PER_CORE, 512], bf16,
                                        tag="attnT")
                nj = 4 * c + 4

                def emit_s(h, j):
                    """S^T matmul + exp for (head h, key tile j); diagonal
                    blocks get the mask added by a tiny extra matmul in the
                    same accumulation group (identity.T @ mask = mask), so
                    the S->exp chain never leaves the PE/ACT engines."""
                    o = j - 4 * c
                    lo = max(o, 0) * 128
                    q0 = c * 512
                    s_ps = psum.tile([128, 512], f32, tag="A", bufs=4,
                                     name="s_ps")
                    if o >= 0:
                        nc.tensor.matmul(s_ps[:, lo:lo + 128],
                                         kT[:, ts(j, 128)],
                                         qT[:, h, q0 + lo:q0 + lo + 128],
                                         start=True, stop=False)
                        nc.tensor.matmul(s_ps[:, lo:lo + 128], ident_sb[:],
                                         mask_sb[:], start=False, stop=True)
                        if lo + 128 < 512:
                            nc.tensor.matmul(s_ps[:, lo + 128:],
                                             kT[:, ts(j, 128)],
                                             qT[:, h, q0 + lo + 128:q0 + 512],
                                             start=True, stop=True)
                    else:
                        nc.tensor.matmul(s_ps[:], kT[:, ts(j, 128)],
                                         qT[:, h, q0:q0 + 512],
                                         start=True, stop=True)
                    p = p_pool.tile([128, 512], bf16, tag="p")
                    nc.scalar.activation(p[:, lo:], s_ps[:, lo:], Exp,
                                         bias=0.0, scale=SCALE)
                    return p, lo

                # late input loads on the scalar queue, issued while this
                # chunk's attention runs: wo (needed from wo_chunk(0)) and
                # the xT columns / rope tables of chunk c+1.
                if c == 0:
                    for h in range(HEADS_PER_CORE):
                        nc.scalar.dma_start(wo[:, h, :], wo_d[:, h, :])
                if c + 1 < NCQ:
                    ns = slice((c + 1) * 512, (c + 2) * 512)
                    nc.scalar.dma_start(xT[:, :, ns], xT_d[:, :, ns])
                    nc.scalar.dma_start(cos_sb[:, ns], cos_d[:, ns])
                    nc.scalar.dma_start(ssig_sb[:, ns], ssig_d[:, ns])
                for h in range(HEADS_PER_CORE):
                    out_ps = psum.tile([128, 512], f32, tag="B", bufs=4)
                    sums_ps = psum.tile([128, 512], f32, tag="B", bufs=4)
                    pend = [emit_s(h, j) for j in range(min(2, nj))]
                    for j in range(nj):
                        if j + 2 < nj:
                            pend.append(emit_s(h, j + 2))
                        p, lo = pend.pop(0)
                        nc.tensor.matmul(out_ps[:, lo:], v_sb[:, j, :],
                                         p[:, lo:],
                                         start=(j == 0), stop=(j == nj - 1))
                        nc.tensor.matmul(sums_ps[:, lo:], ones_sb[:],
                                         p[:, lo:],
                                         start=(j == 0), stop=(j == nj - 1))
                    rc = recip_pool.tile([128, 512], f32, tag="rc")
                    nc.vector.reciprocal_approx_fast(out=rc[:], in_=sums_ps[:])
                    nc.vector.tensor_mul(attn_t[:, h, :], out_ps[:], rc[:])
                return attn_t

            def wo_chunk(c, attn_t):
                """Output projection for q chunk c; bf16 partials to DRAM."""
                for tq in range(4):
                    row0 = (4 * c + tq) * 128
                    for nn in range(4):
                        yp = psum.tile([128, 512], f32, tag="A", bufs=4)
                        for h in range(HEADS_PER_CORE):
                            nc.tensor.matmul(yp[:],
                                             attn_t[:, h, ts(tq, 128)],
                                             wo[:, h, ts(nn, 512)],
                                             start=(h == 0), stop=(h == 3))
                        ysb = y_pool.tile([128, 512], bf16, tag="y")
                        if nn % 2 == 0:
                            nc.scalar.copy(ysb[:], yp[:])
                        else:
                            nc.vector.tensor_copy(ysb[:], yp[:])
                        nc.sync.dma_start(
                            y_d[row0:row0 + 128, ts(nn, 512)], ysb[:])

            # Phase order p0 a0 [p1] w0 a1 [p2] w1 a2 [p3] w2 a3 w3:
            # proj(c+1) has no dependency on attn(c), so emitting it before
            # wo(c) gives the PE ~20us of independent work while the last
            # head's softmax normalization (ACT exp tail + DVE recip/mul)
            # completes -- wo(c) then starts with attn_t fully ready.
            proj_chunk(0)
            for c in range(NCQ):
                at = attn_chunk(c)
                if c + 1 < NCQ:
                    proj_chunk(c + 1)
                wo_chunk(c, at)

    nc.finalize()
    return nc


def _to_pmajor(a, nk):
    """[nk*128, F] -> [128, nk, F] partition-major layout."""
    f = a.shape[1]
    return np.ascontiguousarray(a.reshape(nk, 128, f).transpose(1, 0, 2))


def _prep_inputs(x, Wq, Wk, Wv, Wo, t_len=T):
    """Host-side shard + layout prep -> per-core input maps."""
    import ml_dtypes
    bf16 = ml_dtypes.bfloat16

    x = np.asarray(x, np.float32)
    Wq = np.asarray(Wq, np.float32)
    Wk = np.asarray(Wk, np.float32)
    Wv = np.asarray(Wv, np.float32)
    Wo = np.asarray(Wo, np.float32)

    NK = D // 128

    # RoPE de-interleave permutation within one head: [evens | odds]
    perm = np.concatenate([np.arange(0, HD, 2), np.arange(1, HD, 2)])

    # rope tables (match reference: freqs = t * base**(-2j/HD))
    inv = 1.0 / (ROPE_BASE ** (np.arange(0, HD, 2, dtype=np.float32) / HD))
    tpos = np.arange(t_len, dtype=np.float32)
    f = inv[:, None] * tpos[None, :]                       # [64, T]
    cos_dup = np.concatenate([np.cos(f), np.cos(f)], 0)    # [128, T]
    ssig = np.concatenate([-np.sin(f), np.sin(f)], 0)      # [128, T]
    cos_dup = cos_dup.astype(bf16)
    ssig = ssig.astype(bf16)

    # strict-lower-triangular causal mask template for the diagonal
    # [tk-tile, tq-tile] block (tk > tq within the 128x128 block)
    r = np.arange(128)[:, None]
    col = np.arange(128)[None, :]
    mask_t = np.where(r > col, MASK_VAL, 0.0).astype(bf16)
    ident = np.eye(128, dtype=np.float32).astype(bf16)

    in_maps = []
    for b in range(B):
        xT_b = _to_pmajor(np.ascontiguousarray(x[b, :t_len].T),
                          NK).astype(bf16)               # [128, NK, T]
        for g in range(KV):
            wq_g = Wq[:, g * DQ:(g + 1) * DQ].reshape(D, HEADS_PER_CORE, HD)
            wq_g = _to_pmajor(np.ascontiguousarray(
                wq_g[:, :, perm].reshape(D, DQ)), NK).astype(bf16)
            wk_g = _to_pmajor(np.ascontiguousarray(
                Wk[:, g * HD:(g + 1) * HD][:, perm]), NK).astype(bf16)
            wv_g = _to_pmajor(np.ascontiguousarray(
                Wv[:, g * HD:(g + 1) * HD]), NK).astype(bf16)
            wo_g = _to_pmajor(np.ascontiguousarray(
                Wo[g * DQ:(g + 1) * DQ, :]), HEADS_PER_CORE).astype(bf16)
            in_maps.append({
                "xT": xT_b, "wq": wq_g, "wk": wk_g, "wv": wv_g,
                "wo": wo_g, "cosd": cos_dup, "ssig": ssig, "mask": mask_t,
                "ident": ident,
            })
    return in_maps


def run(inputs, trace=False, t_len=T):
    """Run the sharded kernel; returns (y_full, BassKernelResults)."""
    from concourse.bass_utils import run_bass_kernel_spmd

    key = ("nc", t_len)
    if key not in _CACHE:
        _CACHE[key] = _build_nc(t_len)
    nc = _CACHE[key]

    in_maps = _prep_inputs(inputs["x"], inputs["Wq"], inputs["Wk"],
                           inputs["Wv"], inputs["Wo"], t_len)
    res = run_bass_kernel_spmd(nc, in_maps, list(range(N_CORES)), trace=trace)

    y = np.empty((B, t_len, D), np.float32)
    for b in range(B):
        acc = np.zeros((t_len, D), np.float32)
        for g in range(KV):
            acc += np.asarray(res.results[b * KV + g]["y"], np.float32)
        y[b] = acc
    return y, res


def kernel(**inputs) -> np.ndarray:
    y, _ = run(inputs, trace=False)
    return y
